# revision 14
# baseline (speedup 1.0000x reference)
"""CAVPCore Trainium2 kernel: 2 LSTM cells + 4 Bahdanau attention heads.

Sharding (8 NeuronCores, single NEFF, 3 small collectives):
  - Visual & language LSTM: gate-dim sharded 8-way. Core k computes the
    128-wide slice k of h/c for the full batch B=256, holding 1/8 of the
    LSTM weights. h_vis is AllGather'd (for the language LSTM input) and
    AllToAll'd (to hand each core the full-R h_vis for its batch rows).
  - Attention: batch-sharded, 32 rows/core. output_feats AllGather'd back
    for the language LSTM.
All DRAM-side layout transforms (transposes, gate reordering, bias folding)
are free host-side numpy; the device sees matmul-ready layouts (contraction
dim on partitions everywhere).

Self-contained: hardcodes shapes from the problem spec.
"""

import os

import numpy as np

import concourse.bass as bass
import concourse.mybir as mybir
import concourse.tile as tile
from concourse import bacc
from concourse.bass_utils import run_bass_kernel_spmd

F32 = mybir.dt.float32
BF16 = mybir.dt.bfloat16
AF = mybir.ActivationFunctionType
OP = mybir.AluOpType
AX = mybir.AxisListType

B, L, R, A, E = 256, 64, 1024, 512, 1024
NC = 8
BLOC = B // NC          # 32 batch rows per core (attention)
RS = R // NC            # 128 hidden slice per core (LSTM)
BLL = BLOC * L          # 2048 attention rows per core
KV = E + 2 * R          # 3072 visual LSTM input width
KL = 2 * R              # 2048 language LSTM input width

# precision knobs: "f32" | "bf16"
DT_LSTM_S = os.environ.get("CAVP_DT_LSTM", "f32")
DT_ATT_S = os.environ.get("CAVP_DT_ATT", "f32")
DT_LSTM = BF16 if DT_LSTM_S == "bf16" else F32
DT_ATT = BF16 if DT_ATT_S == "bf16" else F32
STAGE = int(os.environ.get("CAVP_STAGE", "70"))


def _np_dt(d):
    import ml_dtypes
    return ml_dtypes.bfloat16 if d == BF16 else np.float32


def _t(x):
    return np.ascontiguousarray(np.transpose(np.asarray(x, dtype=np.float32)))


def _gate_rows(k):
    return np.concatenate(
        [np.arange(g * R + k * RS, g * R + (k + 1) * RS) for g in range(4)])


# name -> (shape, dtype-group): "L"=DT_LSTM, "A"=DT_ATT, "F"=f32
INPUT_SPECS = {
    "envT": ((KV, B), "L"), "h0T": ((R, B), "L"),
    "ones_row": ((1, 128), "A"), "masks": ((128, 2), "A"),
    "selC": ((32, 16, 128), "A"), "sel3": ((32, 96), "A"), "ident": ((128, 128), "F"),
    "sp_whT": ((R, A), "A"), "ctx_whT": ((R, A), "A"),
    "cmp_whT": ((R, A), "A"), "out_whT": ((R, A), "A"),
    "ctx_wvT": ((R, A), "A"), "cmp_wvT": ((R, A), "A"), "out_wvT": ((R, A), "A"),
    "sp_waRep": ((128, A), "A"), "ctx_waRep": ((128, A), "A"),
    "cmp_waRep": ((128, A), "A"), "out_waRep": ((128, A), "A"),
    "sp_bias": ((1, A), "A"), "ctx_bias": ((1, A), "A"),
    "cmp_bias": ((1, A), "A"), "out_bias": ((1, A), "A"),
    "WvT": ((KV, 4 * RS), "L"), "WhvT": ((R, 4 * RS), "L"),
    "bv": ((128, 4), "F"), "c0T": ((RS, B), "F"),
    "WlT": ((KL, 4 * RS), "L"), "WhlT": ((R, 4 * RS), "L"),
    "bl": ((128, 4), "F"), "c1T": ((RS, B), "F"),
    "p_roi": ((BLL, A), "A"), "roiT": ((R, BLL), "A"), "roi_nat": ((BLL, R), "A"),
    "ctxT": ((R, BLL), "A"), "ctx_nat": ((BLL, R), "A"), "fcT": ((R, BLOC), "A"),
}


def prep_inputs(inp):
    """Build per-core input maps (host-side numpy, free wrt HW exec time)."""
    grp_np = {"L": _np_dt(DT_LSTM), "A": _np_dt(DT_ATT), "F": np.float32}

    shared = {}
    shared["envT"] = np.concatenate(
        [_t(inp["state_h"][1]), _t(inp["fc_feats"]), _t(inp["xt"])], axis=0)
    shared["h0T"] = _t(inp["state_h"][0])
    shared["ones_row"] = np.ones((1, 128), dtype=np.float32)
    masks = np.zeros((128, 2), dtype=np.float32)
    masks[:64, 0] = 1.0
    masks[64:, 1] = 1.0
    shared["masks"] = masks
    selC = np.zeros((32, 16, 128), dtype=np.float32)
    for c in range(16):
        selC[2 * c, c, :64] = 1.0
        selC[2 * c + 1, c, 64:] = 1.0
    shared["selC"] = selC
    sel3 = np.zeros((32, 96), dtype=np.float32)
    for b in range(32):
        sel3[b, 3 * b:3 * b + 3] = 1.0
    shared["sel3"] = sel3
    shared["ident"] = np.eye(128, dtype=np.float32)
    for nm in ("sp_wh", "ctx_wh", "cmp_wh", "out_wh", "ctx_wv", "cmp_wv", "out_wv"):
        shared[nm + "T"] = _t(inp[nm + "_w"])
    for nm in ("sp", "ctx", "cmp", "out"):
        shared[nm + "_waRep"] = np.tile(
            np.asarray(inp[nm + "_wa_w"], np.float32), (128, 1))
    shared["sp_bias"] = np.asarray(inp["sp_wh_b"], np.float32)[None, :]
    shared["ctx_bias"] = np.asarray(inp["ctx_wh_b"] + inp["ctx_wv_b"], np.float32)[None, :]
    shared["cmp_bias"] = np.asarray(inp["cmp_wh_b"] + inp["cmp_wv_b"], np.float32)[None, :]
    shared["out_bias"] = np.asarray(inp["out_wh_b"] + inp["out_wv_b"], np.float32)[None, :]

    in_maps = []
    for k in range(NC):
        rows = _gate_rows(k)
        bk = slice(k * BLOC, (k + 1) * BLOC)
        m = dict(shared)
        m["WvT"] = _t(np.asarray(inp["W_ih_v"])[rows])
        m["WhvT"] = _t(np.asarray(inp["W_hh_v"])[rows])
        m["bv"] = np.ascontiguousarray(
            np.asarray(inp["b_ih_v"] + inp["b_hh_v"], np.float32)[rows].reshape(4, RS).T)
        m["c0T"] = _t(np.asarray(inp["state_c"])[0][:, k * RS:(k + 1) * RS])
        m["WlT"] = _t(np.asarray(inp["W_ih_l"])[rows])
        m["WhlT"] = _t(np.asarray(inp["W_hh_l"])[rows])
        m["bl"] = np.ascontiguousarray(
            np.asarray(inp["b_ih_l"] + inp["b_hh_l"], np.float32)[rows].reshape(4, RS).T)
        m["c1T"] = _t(np.asarray(inp["state_c"])[1][:, k * RS:(k + 1) * RS])
        m["p_roi"] = np.asarray(inp["p_roi_feats"], np.float32)[bk].reshape(BLL, A)
        roi = np.asarray(inp["roi_feats"], np.float32)[bk].reshape(BLL, R)
        m["roiT"] = _t(roi)
        m["roi_nat"] = roi
        ctx = np.asarray(inp["context_feats"], np.float32)[bk].reshape(BLL, R)
        m["ctxT"] = _t(ctx)
        m["ctx_nat"] = ctx
        m["fcT"] = _t(np.asarray(inp["fc_feats"])[bk])
        m = {n: np.ascontiguousarray(v.astype(grp_np[INPUT_SPECS[n][1]]))
             for n, v in m.items()}
        in_maps.append(m)
    return in_maps


def build():
    nc = bacc.Bacc("TRN2", target_bir_lowering=False, debug=False, num_devices=NC)
    grp_dt = {"L": DT_LSTM, "A": DT_ATT, "F": F32}
    din = {n: nc.dram_tensor(n, list(shape), grp_dt[g], kind="ExternalInput")
           for n, (shape, g) in INPUT_SPECS.items()}

    out_hv = nc.dram_tensor("out_hv", [RS, B], F32, kind="ExternalOutput")
    out_cv = nc.dram_tensor("out_cv", [RS, B], F32, kind="ExternalOutput")
    out_hl = nc.dram_tensor("out_hl", [RS, B], F32, kind="ExternalOutput")
    out_cl = nc.dram_tensor("out_cl", [RS, B], F32, kind="ExternalOutput")
    out_of = nc.dram_tensor("out_of", [R, BLOC], F32, kind="ExternalOutput")
    dbg = nc.dram_tensor("dbg", [128, 1024], F32, kind="ExternalOutput")

    with tile.TileContext(nc) as tc:
        with (
            tc.tile_pool(name="sb", bufs=1) as sb,
            tc.tile_pool(name="sbw", bufs=3) as sbw,
            tc.tile_pool(name="ps", bufs=8, space="PSUM") as ps,
            tc.tile_pool(name="dr", bufs=1, space="DRAM") as dr,
        ):
            _emit(nc, tc, din, sb, sbw, ps, dr,
                  out_hv, out_cv, out_hl, out_cl, out_of, dbg)
    nc.compile()
    return nc


def _lstm_gates(nc, sb, sbw, ps, name, WT_d, WhT_d, bias_d, cT_d, rhs_w, rhs_h):
    """One gate-sharded LSTM cell slice -> (hT, cT) f32 sbuf tiles [128, 256]."""
    nkw = WT_d.shape[0] // 128
    nkh = WhT_d.shape[0] // 128
    gates = [ps.tile([128, 512], F32, tag="bank", name=f"gates_{name}_{g}")
             for g in range(4)]
    for kc in range(nkw):
        wt = sbw.tile([128, 512], DT_LSTM, tag="w_lstm", name=f"w_{name}_{kc}")
        nc.sync.dma_start(wt[:], WT_d[kc * 128:(kc + 1) * 128, :])
        rt = rhs_w(kc)
        for g in range(4):
            nc.tensor.matmul(gates[g][:, :256], wt[:, g * 128:(g + 1) * 128], rt,
                             start=(kc == 0), stop=False)
    for kc in range(nkh):
        wt = sbw.tile([128, 512], DT_LSTM, tag="w_lstm", name=f"wh_{name}_{kc}")
        nc.sync.dma_start(wt[:], WhT_d[kc * 128:(kc + 1) * 128, :])
        rt = rhs_h(kc)
        for g in range(4):
            nc.tensor.matmul(gates[g][:, :256], wt[:, g * 128:(g + 1) * 128], rt,
                             start=False, stop=(kc == nkh - 1))
    bv = sb.tile([128, 4], F32, name=f"bv_{name}")
    nc.sync.dma_start(bv[:], bias_d[:, :])
    c0 = sb.tile([128, 256], F32, name=f"c0_{name}")
    nc.sync.dma_start(c0[:], cT_d[:, :])
    sI = sb.tile([128, 256], F32, name=f"sI_{name}")
    sF = sb.tile([128, 256], F32, name=f"sF_{name}")
    tG = sb.tile([128, 256], F32, name=f"tG_{name}")
    sO = sb.tile([128, 256], F32, name=f"sO_{name}")
    nc.scalar.activation(sI[:], gates[0][:, :256], AF.Sigmoid, bias=bv[:, 0:1])
    nc.scalar.activation(sF[:], gates[1][:, :256], AF.Sigmoid, bias=bv[:, 1:2])
    nc.scalar.activation(tG[:], gates[2][:, :256], AF.Tanh, bias=bv[:, 2:3])
    nc.scalar.activation(sO[:], gates[3][:, :256], AF.Sigmoid, bias=bv[:, 3:4])
    cT = sb.tile([128, 256], F32, name=f"cT_{name}")
    tmp = sb.tile([128, 256], F32, name=f"tmp_{name}")
    nc.vector.tensor_mul(cT[:], sF[:], c0[:])
    nc.vector.tensor_mul(tmp[:], sI[:], tG[:])
    nc.vector.tensor_add(cT[:], cT[:], tmp[:])
    tc2 = sb.tile([128, 256], F32, name=f"tc2_{name}")
    nc.scalar.activation(tc2[:], cT[:], AF.Tanh)
    hT = sb.tile([128, 256], F32, name=f"hT_{name}")
    nc.vector.tensor_mul(hT[:], sO[:], tc2[:])
    return hT, cT


def _emit(nc, tc, din, sb, sbw, ps, dr, out_hv, out_cv, out_hl, out_cl, out_of, dbg):
    rg = [list(range(NC))]

    fills = {"out_hl": out_hl, "out_cl": out_cl}

    def finish(stage, dbg_ap=None):
        """Zero-fill outputs not yet written when stopping at `stage`."""
        if STAGE > stage:
            return False
        z = sb.tile([128, 1024], F32, name=f"zfill_{stage}")
        nc.vector.memset(z[:], 0.0)
        if dbg_ap is not None:
            nc.vector.tensor_copy(z[:dbg_ap.shape[0], :dbg_ap.free_size()],
                                  dbg_ap)
        nc.sync.dma_start(dbg[:, :], z[:])
        for name, t in fills.items():
            nc.sync.dma_start(t[:, :], z[:t.shape[0], :t.shape[1]])
        if "out_of" in fills or STAGE <= 60:
            nc.sync.dma_start(
                out_of.ap().rearrange("(c p) f -> p c f", p=128),
                z[:].rearrange("p (c f) -> p c f", c=8)[:, :, :BLOC])
        return True

    # ---------------- small resident constants ----------------------------
    ones_row = sb.tile([1, 128], DT_ATT)
    nc.sync.dma_start(ones_row[:], din["ones_row"][:, :])
    masks = sb.tile([128, 2], DT_ATT)
    nc.sync.dma_start(masks[:], din["masks"][:, :])
    selC = sb.tile([32, 16, 128], DT_ATT)
    nc.sync.dma_start(selC[:], din["selC"][:, :, :])
    sel3 = sb.tile([32, 96], DT_ATT)
    nc.sync.dma_start(sel3[:], din["sel3"][:, :])
    ident = sb.tile([128, 128], F32)
    nc.sync.dma_start(ident[:], din["ident"][:, :])
    waRep = {}
    for h in ("sp", "ctx", "cmp", "out"):
        w = sb.tile([128, A], DT_ATT, name=f"waRep_{h}")
        nc.sync.dma_start(w[:], din[h + "_waRep"][:, :])
        waRep[h] = w
    bias_row = {}
    for h in ("sp", "ctx", "cmp", "out"):
        bw = sb.tile([1, A], DT_ATT, name=f"bias_{h}")
        nc.sync.dma_start(bw[:], din[h + "_bias"][:, :])
        bias_row[h] = bw

    # ---------------- visual LSTM (gate-sharded, full batch) --------------
    def env_tile(kc):
        t = sbw.tile([128, 256], DT_LSTM, tag="rhs_lstm", name=f"envT_{kc}")
        nc.sync.dma_start(t[:], din["envT"][kc * 128:(kc + 1) * 128, :])
        return t[:]

    def h0_tile(kc):
        t = sbw.tile([128, 256], DT_LSTM, tag="rhs_lstm", name=f"h0T_{kc}")
        nc.sync.dma_start(t[:], din["h0T"][kc * 128:(kc + 1) * 128, :])
        return t[:]

    hvT, cvT = _lstm_gates(nc, sb, sbw, ps, "v", din["WvT"], din["WhvT"],
                           din["bv"], din["c0T"], env_tile, h0_tile)
    nc.sync.dma_start(out_cv[:, :], cvT[:])
    nc.sync.dma_start(out_hv[:, :], hvT[:])

    if DT_LSTM != F32:
        hv_mm = sb.tile([128, 256], DT_LSTM)
        nc.scalar.activation(hv_mm[:], hvT[:], AF.Copy)
    else:
        hv_mm = hvT

    ag1_in = dr.tile([RS, B], DT_LSTM)
    ag1_out = dr.tile([NC, RS, B], DT_LSTM)
    nc.sync.dma_start(ag1_in[:], hv_mm[:])
    nc.gpsimd.collective_compute("AllGather", OP.bypass, replica_groups=rg,
                                 ins=[ag1_in.opt()], outs=[ag1_out.opt()])

    ata_in = dr.tile([NC, RS, BLOC], DT_LSTM)
    ata_out = dr.tile([NC, RS, BLOC], DT_LSTM)
    nc.sync.dma_start(
        ata_in[:, :, :].transpose([1, 0, 2]),
        hv_mm.rearrange("p (c f) -> p c f", c=NC))
    nc.gpsimd.collective_compute("AllToAll", OP.bypass, replica_groups=rg,
                                 ins=[ata_in.opt()], outs=[ata_out.opt()])

    # h_visT full-R for my 32 batch rows: [128, 8, 32] (partition = r % 128)
    hv_sb_l = sb.tile([128, NC, BLOC], DT_LSTM)
    nc.sync.dma_start(hv_sb_l[:], ata_out[:, :, :].transpose([1, 0, 2]))
    if finish(10, hv_sb_l[:].rearrange("p c f -> p (c f)")):
        return
    if DT_ATT != DT_LSTM:
        hv_sb = sb.tile([128, NC, BLOC], DT_ATT)
        nc.scalar.activation(hv_sb[:], hv_sb_l[:], AF.Copy)
    else:
        hv_sb = hv_sb_l

    # ---------------- hp per head: [32, 512] = h @ whT + bias -------------
    hp_sb = {}
    for h in ("sp", "ctx", "cmp", "out"):
        hp_ps = ps.tile([32, A], F32, tag="bank", name=f"hp_ps_{h}")
        for kc in range(8):
            wt = sbw.tile([128, A], DT_ATT, tag="whT", name=f"whT_{h}_{kc}")
            nc.sync.dma_start(wt[:], din[h + "_whT"][kc * 128:(kc + 1) * 128, :])
            nc.tensor.matmul(hp_ps[:], hv_sb[:, kc, :], wt[:],
                             start=(kc == 0), stop=False)
        nc.tensor.matmul(hp_ps[:], ones_row[:, :32], bias_row[h][:],
                         start=False, stop=True)
        t = sb.tile([32, A], DT_ATT, name=f"hp_sb_{h}")
        nc.scalar.activation(t[:], hp_ps[:], AF.Copy)
        hp_sb[h] = t
    if STAGE <= 20:
        hpf = sb.tile([32, A], F32, name="hp_f32")
        nc.vector.tensor_copy(hpf[:], hp_sb["ctx"][:])
        finish(20, hpf[:])
        return

    # ---------------- attention shared pieces ------------------------------
    def softmax16(logits, name, diag_writes):
        """logits [128,16] f32 -> per-b recip-bcast rb_sb [128,32] f32.
        diag_writes: list of (flat_tile, offset, period): write masked exp at
        cols offset + j + c*period (the zero-padded value-matmul rhs)."""
        Ex = sb.tile([128, 16], DT_ATT, name=f"E_{name}")
        nc.scalar.activation(Ex[:], logits[:], AF.Exp)
        for (tf, off, per) in diag_writes:
            nc.vector.tensor_scalar_mul(tf[:, off::per], Ex[:], masks[:, 0:1])
            nc.vector.tensor_scalar_mul(tf[:, off + 1::per], Ex[:], masks[:, 1:2])
        s0 = ps.tile([1, 16], F32, tag="bank", name=f"s0_{name}")
        nc.tensor.matmul(s0[:], masks[:, 0:1], Ex[:], start=True, stop=True)
        s1 = ps.tile([1, 16], F32, tag="bank", name=f"s1_{name}")
        nc.tensor.matmul(s1[:], masks[:, 1:2], Ex[:], start=True, stop=True)
        rrow = sb.tile([1, 32], F32, name=f"rrow_{name}")
        rv = rrow.rearrange("p (c j) -> p c j", j=2)
        nc.vector.reciprocal(rv[:, :, 0], s0[:])
        nc.vector.reciprocal(rv[:, :, 1], s1[:])
        if DT_ATT != F32:
            rrow_mm = sb.tile([1, 32], DT_ATT, name=f"rrow_mm_{name}")
            nc.scalar.activation(rrow_mm[:], rrow[:], AF.Copy)
        else:
            rrow_mm = rrow
        rb_ps = ps.tile([128, 32], F32, tag="bank", name=f"rb_ps_{name}")
        nc.tensor.matmul(rb_ps[:], ones_row[:], rrow_mm[:], start=True, stop=True)
        rb = sb.tile([128, 32], F32, name=f"rb_{name}")
        nc.scalar.activation(rb[:], rb_ps[:], AF.Copy)
        return rb

    # zero-padded value-matmul rhs: ebig2 [128, 16*(32 sp | 32 cmp)], ebig_ctx
    ebig2 = sb.tile([128, 1024], DT_ATT)
    nc.vector.memset(ebig2[:], 0.0)
    ebig_ctx = sb.tile([128, 512], DT_ATT)
    nc.vector.memset(ebig_ctx[:], 0.0)

    # ---------------- SingleSP logits (p_roi precomputed proj) -------------
    lg_sp = sb.tile([128, 16], F32)
    for c in range(16):
        proi_t = sbw.tile([128, A], DT_ATT, tag="proi", name=f"proi_{c}")
        nc.sync.dma_start(proi_t[:], din["p_roi"][c * 128:(c + 1) * 128, :])
        pre_ps = ps.tile([128, A], F32, tag="bank", name=f"presp_{c}")
        nc.tensor.matmul(pre_ps[:], selC[:, c, :], hp_sb["sp"][:],
                         start=True, stop=True)
        pre_sb = sbw.tile([128, A], F32, tag="pre_sb", bufs=2, name=f"presb_{c}")
        nc.vector.tensor_add(pre_sb[:], pre_ps[:], proi_t[:])
        dot = sbw.tile([128, A], DT_ATT, tag="dot", name=f"dotsp_{c}")
        nc.scalar.activation(dot[:], pre_sb[:], AF.Tanh)
        scr = sbw.tile([128, A], F32, tag="scr", bufs=2, name=f"scr_sp_{c}")
        nc.vector.tensor_mul(scr[:], dot[:], waRep["sp"][:])
        nc.vector.reduce_sum(lg_sp[:, c:c + 1], scr[:], axis=AX.X)

    if finish(30, lg_sp[:]):
        return

    # ---------------- ContextSP logits (proj on the fly) -------------------
    wv_ctx = sb.tile([128, 8, A], DT_ATT)
    wv_cmp = sb.tile([128, 8, A], DT_ATT)
    wv_out = sb.tile([128, 8, A], DT_ATT)
    for rc in range(8):
        nc.sync.dma_start(wv_ctx[:, rc, :], din["ctx_wvT"][rc * 128:(rc + 1) * 128, :])
        nc.sync.dma_start(wv_cmp[:, rc, :], din["cmp_wvT"][rc * 128:(rc + 1) * 128, :])
        nc.sync.dma_start(wv_out[:, rc, :], din["out_wvT"][rc * 128:(rc + 1) * 128, :])

    lg_ctx = sb.tile([128, 16], F32)
    for cg in range(4):
        big = sbw.tile([128, 8, 512], DT_ATT, tag="bigT", bufs=2, name=f"ctxT_{cg}")
        for rc in range(8):
            nc.sync.dma_start(
                big[:, rc, :],
                din["ctxT"][rc * 128:(rc + 1) * 128, cg * 512:(cg + 1) * 512])
        for ci in range(4):
            c = cg * 4 + ci
            pre_ps = ps.tile([128, A], F32, tag="bank", name=f"prectx_{c}")
            for rc in range(8):
                nc.tensor.matmul(pre_ps[:], big[:, rc, ci * 128:(ci + 1) * 128],
                                 wv_ctx[:, rc, :], start=(rc == 0), stop=False)
            nc.tensor.matmul(pre_ps[:], selC[:, c, :], hp_sb["ctx"][:],
                             start=False, stop=True)
            dot = sbw.tile([128, A], DT_ATT, tag="dot", name=f"dotctx_{c}")
            nc.scalar.activation(dot[:], pre_ps[:], AF.Tanh)
            scr = sbw.tile([128, A], F32, tag="scr", bufs=2, name=f"scr_ctx_{c}")
            nc.vector.tensor_mul(scr[:], dot[:], waRep["ctx"][:])
            nc.vector.reduce_sum(lg_ctx[:, c:c + 1], scr[:], axis=AX.X)

    if finish(31, lg_ctx[:]):
        return
    rb_ctx = softmax16(lg_ctx, "ctx", [(ebig_ctx, 0, 34)])
    if finish(32, rb_ctx[:]):
        return

    # ---------------- ContextSP value sum -> cfT ---------------------------
    NRC = int(os.environ.get("CAVP_NRC", "8"))
    feat_ctx = [ps.tile([128, 32], F32, tag="bank", name=f"fctx_{rc}")
                for rc in range(NRC)]
    ebig_ctx_v = ebig_ctx.rearrange("p (c n) -> p c n", n=32)
    for c in range(16):
        nat = sbw.tile([128, R], DT_ATT, tag="natT", name=f"ctxnat_{c}")
        nc.sync.dma_start(nat[:], din["ctx_nat"][c * 128:(c + 1) * 128, :])
        for rc in range(NRC):
            nc.tensor.matmul(feat_ctx[rc][:, :],
                             nat[:, rc * 128:(rc + 1) * 128], ebig_ctx_v[:, c, :],
                             start=(c == 0), stop=(c == 15))
    if STAGE == 33:
        finish(33, ebig_ctx[:, :])
        return
    if STAGE == 34:
        f0 = sb.tile([128, 32], F32, name="f0dbg")
        nc.vector.tensor_copy(f0[:], feat_ctx[0][:, :])
        finish(34, f0[:])
        return
    cfT = sb.tile([128, 8, BLOC], F32)
    nc.vector.memset(cfT[:], 0.0)
    for rc in range(NRC):
        nc.vector.tensor_mul(cfT[:, rc, :], feat_ctx[rc][:, :], rb_ctx[:])
    if finish(40, cfT[:].rearrange("p c f -> p (c f)")):
        return

    # ---------------- CompSP logits (rel built on the fly) -----------------
    lg_cmp = sb.tile([128, 16], F32)
    for cg in range(4):
        big = sbw.tile([128, 8, 512], DT_ATT, tag="bigT", bufs=2, name=f"roiT_{cg}")
        for rc in range(8):
            nc.sync.dma_start(
                big[:, rc, :],
                din["roiT"][rc * 128:(rc + 1) * 128, cg * 512:(cg + 1) * 512])
        for ci in range(4):
            c = cg * 4 + ci
            pre_ps = ps.tile([128, A], F32, tag="bank", name=f"precmp_{c}")
            for rc in range(8):
                relT = sbw.tile([128, 128], DT_ATT, tag="relT", name=f"relT_{c}_{rc}")
                nc.vector.tensor_tensor(
                    relT.rearrange("p (b l) -> p b l", b=2),
                    cfT[:, rc, 2 * c:2 * c + 2].unsqueeze(2).broadcast_to([128, 2, 64]),
                    big[:, rc, ci * 128:(ci + 1) * 128].rearrange(
                        "p (b l) -> p b l", b=2),
                    op=OP.subtract)
                nc.tensor.matmul(pre_ps[:], relT[:], wv_cmp[:, rc, :],
                                 start=(rc == 0), stop=False)
            nc.tensor.matmul(pre_ps[:], selC[:, c, :], hp_sb["cmp"][:],
                             start=False, stop=True)
            dot = sbw.tile([128, A], DT_ATT, tag="dot", name=f"dotcmp_{c}")
            nc.scalar.activation(dot[:], pre_ps[:], AF.Tanh)
            scr = sbw.tile([128, A], F32, tag="scr", bufs=2, name=f"scr_cmp_{c}")
            nc.vector.tensor_mul(scr[:], dot[:], waRep["cmp"][:])
            nc.vector.reduce_sum(lg_cmp[:, c:c + 1], scr[:], axis=AX.X)

    rb_sp = softmax16(lg_sp, "sp", [(ebig2, 0, 66)])
    rb_cmp = softmax16(lg_cmp, "cmp", [(ebig2, 32, 66)])

    # ---------------- roi value sums (SingleSP + CompSP at once) -----------
    feat_b = [ps.tile([128, 64], F32, tag="bank", name=f"fb_{rc}")
              for rc in range(8)]
    ebig2_v = ebig2.rearrange("p (c n) -> p c n", n=64)
    for c in range(16):
        nat = sbw.tile([128, R], DT_ATT, tag="natT", name=f"roinat_{c}")
        nc.sync.dma_start(nat[:], din["roi_nat"][c * 128:(c + 1) * 128, :])
        for rc in range(8):
            nc.tensor.matmul(feat_b[rc][:, :],
                             nat[:, rc * 128:(rc + 1) * 128], ebig2_v[:, c, :],
                             start=(c == 0), stop=(c == 15))
    sfT = sb.tile([128, 8, BLOC], DT_ATT)
    cmpT = sb.tile([128, 8, BLOC], DT_ATT)
    for rc in range(8):
        nc.vector.tensor_mul(sfT[:, rc, :], feat_b[rc][:, :32], rb_sp[:])
        t = sb.tile([128, BLOC], F32, tag="cmptmp", name=f"cmptmp_{rc}", bufs=2)
        nc.vector.tensor_mul(t[:], feat_b[rc][:, 32:], rb_cmp[:])
        nc.vector.tensor_sub(cmpT[:, rc, :], cfT[:, rc, :], t[:])

    if STAGE <= 50:
        sff = sb.tile([128, 8, BLOC], F32, name="sf_f32")
        nc.vector.tensor_copy(sff[:], sfT[:])
        finish(50, sff[:].rearrange("p c f -> p (c f)"))
        return

    # ---------------- OutputSP --------------------------------------------
    fcT = sb.tile([128, 8, BLOC], DT_ATT)
    nc.sync.dma_start(fcT[:], din["fcT"].ap().rearrange("(c p) f -> p c f", p=128))
    f3 = sb.tile([128, 8, 96], DT_ATT)
    for rc in range(8):
        v = f3.rearrange("p r (b j) -> p r b j", j=3)
        nc.vector.tensor_copy(v[:, rc, :, 0], sfT[:, rc, :])
        nc.vector.tensor_copy(v[:, rc, :, 1], cmpT[:, rc, :])
        nc.vector.tensor_copy(v[:, rc, :, 2], fcT[:, rc, :])
    pre3 = ps.tile([96, A], F32, tag="bank")
    for rc in range(8):
        nc.tensor.matmul(pre3[:], f3[:, rc, :], wv_out[:, rc, :],
                         start=(rc == 0), stop=False)
    nc.tensor.matmul(pre3[:], sel3[:], hp_sb["out"][:], start=False, stop=True)
    dot3 = sbw.tile([96, A], DT_ATT, tag="dot")
    nc.scalar.activation(dot3[:], pre3[:], AF.Tanh)
    lg3 = sb.tile([96, 1], F32)
    scr3 = sbw.tile([128, A], F32, tag="scr", bufs=2, name="scr_out")
    nc.vector.tensor_mul(scr3[:96, :], dot3[:], waRep["out"][:96, :])
    nc.vector.reduce_sum(lg3[:], scr3[:96, :], axis=AX.X)
    tr_ps = ps.tile([1, 96], F32, tag="bank")
    nc.tensor.transpose(tr_ps[:], lg3[:], ident[:96, :96])
    exp3 = sb.tile([1, 96], F32)
    nc.scalar.activation(exp3[:], tr_ps[:], AF.Exp)
    s3 = sb.tile([1, 32], F32)
    nc.vector.reduce_sum(s3[:], exp3.rearrange("p (b j) -> p b j", j=3), axis=AX.X)
    r3 = sb.tile([1, 32], F32)
    nc.vector.reciprocal(r3[:], s3[:])
    w3 = sb.tile([1, 96], DT_ATT)
    nc.vector.tensor_mul(w3.rearrange("p (b j) -> p b j", j=3),
                         exp3.rearrange("p (b j) -> p b j", j=3),
                         r3.unsqueeze(2).broadcast_to([1, 32, 3]))
    wb_ps = ps.tile([128, 96], F32, tag="bank")
    nc.tensor.matmul(wb_ps[:], ones_row[:], w3[:], start=True, stop=True)
    wb = sb.tile([128, 96], F32)
    nc.scalar.activation(wb[:], wb_ps[:], AF.Copy)
    ofT = sb.tile([128, 8, BLOC], F32)
    for rc in range(8):
        p3 = sb.tile([128, 96], F32, tag="p3", name=f"p3_{rc}", bufs=2)
        nc.vector.tensor_mul(p3[:], f3[:, rc, :], wb[:])
        nc.vector.reduce_sum(ofT[:, rc, :],
                             p3.rearrange("p (b j) -> p b j", j=3), axis=AX.X)
    nc.sync.dma_start(out_of.ap().rearrange("(c p) f -> p c f", p=128), ofT[:])
    fills.pop("out_hl"), fills.pop("out_cl")
    if finish(60, ofT[:].rearrange("p c f -> p (c f)")):
        for t in (out_hl, out_cl):
            z = sb.tile([128, 256], F32, name=f"z_{t.name}")
            nc.vector.memset(z[:], 0.0)
            nc.sync.dma_start(t[:, :], z[:])
        return

    if DT_LSTM != F32:
        of_mm = sb.tile([128, 8, BLOC], DT_LSTM)
        nc.scalar.activation(of_mm[:], ofT[:], AF.Copy)
    else:
        of_mm = ofT
    ag2_in = dr.tile([R, BLOC], DT_LSTM)
    ag2_out = dr.tile([NC, R, BLOC], DT_LSTM)
    nc.sync.dma_start(ag2_in[:, :].rearrange("(c p) f -> p c f", p=128), of_mm[:])
    nc.gpsimd.collective_compute("AllGather", OP.bypass, replica_groups=rg,
                                 ins=[ag2_in.opt()], outs=[ag2_out.opt()])

    # ---------------- language LSTM ---------------------------------------
    def lang_rhs(kc):
        if kc < 8:
            t = sbw.tile([128, NC, BLOC], DT_LSTM, tag="rhs_lstm", name=f"ofg_{kc}")
            nc.sync.dma_start(
                t[:], ag2_out[:, kc * 128:(kc + 1) * 128, :].transpose([1, 0, 2]))
            return t.rearrange("p c f -> p (c f)")
        t = sbw.tile([128, 256], DT_LSTM, tag="rhs_lstm", name=f"hvg_{kc}")
        nc.sync.dma_start(t[:], ag1_out[:, :, :][kc - 8])
        return t[:]

    def h1_tile(kc):
        t = sbw.tile([128, 256], DT_LSTM, tag="rhs_lstm", name=f"h1T_{kc}")
        nc.sync.dma_start(t[:], din["envT"][kc * 128:(kc + 1) * 128, :])
        return t[:]

    hlT, clT = _lstm_gates(nc, sb, sbw, ps, "l", din["WlT"], din["WhlT"],
                           din["bl"], din["c1T"], lang_rhs, h1_tile)
    nc.sync.dma_start(out_cl[:, :], clT[:])
    nc.sync.dma_start(out_hl[:, :], hlT[:])
    zd = sb.tile([128, 1024], F32, name="zdbg")
    nc.vector.memset(zd[:], 0.0)
    nc.sync.dma_start(dbg[:, :], zd[:])


_NC_CACHE = None


def kernel(**inputs):
    global _NC_CACHE
    if _NC_CACHE is None:
        _NC_CACHE = build()
    nc = _NC_CACHE
    in_maps = prep_inputs(inputs)
    res = run_bass_kernel_spmd(nc, in_maps, core_ids=list(range(NC))).results
    h_vis = np.concatenate([r["out_hv"] for r in res], axis=0).T
    c_vis = np.concatenate([r["out_cv"] for r in res], axis=0).T
    h_lang = np.concatenate([r["out_hl"] for r in res], axis=0).T
    c_lang = np.concatenate([r["out_cl"] for r in res], axis=0).T
    output_feats = np.concatenate([r["out_of"] for r in res], axis=1).T
    new_h = np.ascontiguousarray(np.stack([h_vis, h_lang]))
    new_c = np.ascontiguousarray(np.stack([c_vis, c_lang]))
    output = np.ascontiguousarray(h_lang)
    return output, new_h, new_c, np.ascontiguousarray(output_feats)
